# revision 26
# baseline (speedup 1.0000x reference)
"""Distributed GAT kernel for Trainium2 (8 NeuronCores, Bass/Tile) — v5.

Sharding: nodes (and incoming edges) are split into 8 contiguous ranges of
6250; graph boundaries align with core boundaries (8 graphs per core), so
pooling is core-local (final risk is per-graph -> concatenated on host).

Design (6.09 ms v1 baseline -> 2.06 ms):
 - h_full rows are 256 BYTES (fp8e4m3 h[128] + fp16 exp(al_src),
   exp(0.2*al_src)) instead of 192 f32 (768 B).  The edge logit uses
   exp(leaky_relu(s+d)) = max(exp(s)exp(d), exp(.2s)exp(.2d)), so the
   exp() moves to phase 0 (per node) and no per-edge activations exist.
 - indicator matrices (Ib / IbT) are built ON CHIP with DVE is_equal from
   tiny fp16 rel-dst tensors instead of 212 MB/layer of f32 DMA.
 - all edge-pipeline matmuls are fp16/fp8 (1 cyc/row vs 4 for f32, fast
   weight loads).
 - edge(l) processing and phase0(l+1) (h = x@Wcat, gate, pooling) are
   fused per 128-node block; AllGather is 7-chunked (tiny last chunk),
   chunk-major, and double-buffered across layers so it overlaps the
   block loop.
 - sigmoid is computed via the exp table (1/(1+exp(-x))) so the scalar
   engine never reloads activation tables.
Remaining bottleneck: dma_gather descriptor generation on the GpSimd/Pool
engine (~5 ns/edge, strictly serial even across SWDGE queues) ~ 1.5 ms
busy, co-limiting with DVE elementwise work (~1.4 ms).  Gathers are
src-sorted for HBM locality and the int16 bucket split (BKT) is aligned
to the AllGather chunk boundary so bucket-A gathers only wait on the
first two chunks of the previous layer's AllGather.
"""
import sys, os, types
sys.path.insert(0, "/opt/trn_rl_repo")
import numpy as np

N = 50000
E = 800000
F = 128
H = 4
C = 32
HC = 128
L = 3
G = 64
OUT = 64
NCORES = 8
NL = N // NCORES          # 6250
NBLK = (NL + 127) // 128  # 49
NPAD = NBLK * 128         # 6272
GL = G // NCORES          # 8
BKT = 24576              # == chunk-major base of AG chunk 2
HGRP = 2
ROWC = 256                # BYTES per h_full row: fp8 h + fp16 AC
WCOL = 145                # Wcat columns
CHUNKS = [(0, 8), (8, 16), (16, 24), (24, 32), (32, 40), (40, 47), (47, 49)]  # AG chunks

_cache = {}
LAST_RESULTS = None


# ----------------------------------------------------------------------
# host-side prep
# ----------------------------------------------------------------------
def _wrap_idxs(idxs):
    idxs = np.asarray(idxs, np.int64)
    n = len(idxs)
    assert n % 128 == 0
    w = idxs.reshape(n // 16, 16).T          # [16, S]
    return np.ascontiguousarray(np.tile(w, (8, 1))).astype(np.int16)


def _remap_rows(n):
    """Global node id -> chunk-major h_full row (see maybe_ag)."""
    n = np.asarray(n, np.int64)
    r0s = np.array([128 * c0 for c0, _ in CHUNKS], np.int64)
    r1s = np.array([min(128 * c1, NL) for _, c1 in CHUNKS], np.int64)
    m, r = n // NL, n % NL
    c = np.searchsorted(r1s, r, side="right")
    return NCORES * r0s[c] + m * (r1s[c] - r0s[c]) + (r - r0s[c])


def _prep(src, dst, batch):
    src = np.asarray(src).astype(np.int64)
    dst = np.asarray(dst).astype(np.int64)
    order = np.argsort(dst, kind="stable")
    src, dst = src[order], dst[order]

    lists = {}
    for m in range(NCORES):
        lo, hi = m * NL, (m + 1) * NL
        i0, i1 = np.searchsorted(dst, [lo, hi])
        s_m, d_m = src[i0:i1], dst[i0:i1] - lo
        blk = d_m >> 7
        bs = np.searchsorted(blk, np.arange(NBLK + 1))
        for b in range(NBLK):
            s_b, d_b = s_m[bs[b]:bs[b + 1]], d_m[bs[b]:bs[b + 1]]
            s_b = _remap_rows(s_b)
            o = np.argsort(s_b, kind="stable")   # ascending rows per gather
            s_b, d_b = s_b[o], d_b[o]
            a = s_b < BKT
            lists[(m, b, 0)] = (s_b[a], d_b[a])
            lists[(m, b, 1)] = (s_b[~a] - BKT, d_b[~a])

    TA = [max((len(lists[(m, b, 0)][0]) + 127) // 128 for m in range(NCORES))
          for b in range(NBLK)]
    TB = [max((len(lists[(m, b, 1)][0]) + 127) // 128 for m in range(NCORES))
          for b in range(NBLK)]

    cores = []
    for m in range(NCORES):
        parts = {k: [] for k in ("idxA", "idxB", "relA", "relB")}
        for b in range(NBLK):
            for T, ik, rk, bk in ((TA[b], "idxA", "relA", 0),
                                  (TB[b], "idxB", "relB", 1)):
                s_b, d_b = lists[(m, b, bk)]
                npad = T * 128 - len(s_b)
                parts[ik].append(np.concatenate(
                    [s_b, np.zeros(npad, np.int64)]))
                parts[rk].append(np.concatenate(
                    [d_b - 128 * b, np.full(npad, 255, np.int64)]))
        cores.append({k: (np.concatenate(v) if v else np.zeros(0, np.int64))
                      for k, v in parts.items()})

    gb = [int(np.ceil(g * N / G)) for g in range(G + 1)]
    bounds = gb[:GL + 1]
    counts = np.diff(np.array(gb)).astype(np.float64)
    bt = np.asarray(batch)
    assert (bt == (np.arange(N) * G // N)).all(), "unexpected batch layout"
    for m in range(NCORES):
        assert gb[m * GL] == m * NL
    return cores, TA, TB, bounds, counts


def _chunk_segments(bounds):
    segs = []
    for k in range(NBLK):
        lo, hi = 128 * k, min(128 * (k + 1), NL)
        out = []
        for g in range(GL):
            s, e = max(lo, bounds[g]), min(hi, bounds[g + 1])
            if s < e:
                out.append((s - lo, e - lo, g))
        segs.append(out)
    return segs


# ----------------------------------------------------------------------
# kernel build
# ----------------------------------------------------------------------
def _build(TA, TB, bounds, cb_zero, debug=False):
    import concourse.bacc as bacc
    import concourse.bass as bass
    import concourse.tile as tile
    import concourse.mybir as mybir
    dt = mybir.dt
    f32 = dt.float32
    f16 = dt.float16
    u8 = dt.uint8
    f8 = dt.float8e4
    AT = mybir.ActivationFunctionType
    OP = mybir.AluOpType
    AX = mybir.AxisListType

    NTA, NTB = sum(TA), sum(TB)
    aoff = np.concatenate([[0], np.cumsum(TA)]).astype(int)
    boff = np.concatenate([[0], np.cumsum(TB)]).astype(int)
    segs = _chunk_segments(bounds)
    groups = [(b0, min(b0 + HGRP, NBLK)) for b0 in range(0, NBLK, HGRP)]

    def bc(sl, *dims):
        """Rebuild the free dims of a sliced AP (keep partition dim)."""
        aps = [list(p) for p in sl.ap]
        return bass.AP(sl.tensor, sl.offset,
                       [aps[0]] + [list(d) for d in dims])

    nc = bacc.Bacc("TRN2", target_bir_lowering=False, debug=False,
                   num_devices=NCORES, num_swdge_queues=4,
                   dynamic_dma_scratch_size=49152)

    def din(name, shape, d=f32):
        return nc.dram_tensor(name, shape, d, kind="ExternalInput")

    x0_d = din("x0", [128, NBLK * 128], f16)
    x0T_d = din("x0T", [128, NBLK * 128], f16)
    idxA_d = din("idxA", [128, NTA * 8], dt.int16)
    idxB_d = din("idxB", [128, NTB * 8], dt.int16)
    rcA_d = din("rcA", [128, NTA], f16)
    rcB_d = din("rcB", [128, NTB], f16)
    rbcA_d = din("rbcA", [128, NTA * 128], f16)
    rbcB_d = din("rbcB", [128, NTB * 128], f16)
    B2_d = din("B2", [128, NBLK * 2], f16)
    rcnt_d = din("rcnt", [128, GL])
    Wcat_d = din("Wcat", [L + 1, 128, WCOL], f16)
    ngb_d = din("ngb", [128, L + 1])
    cb_d = din("cb", [L, 128, 128], f16)
    linW_d = din("linW", [L + 1, 128, OUT])
    linb_d = din("linb", [1, (L + 1) * OUT])
    hw_d = din("hw", [128, L + 1])
    pw_d = din("pw", [128, 3])
    beta_d = din("beta", [64, 1])
    betar_d = din("betar", [1, 64])
    h0_d = din("h0", [1, 1])
    iota_d = din("iota", [128, 128], f16)
    iotaT_d = din("iotaT", [128, 128], f16)
    ident16_d = din("ident16", [128, 128], f16)
    ident8_d = din("ident8", [8, 8])
    onescol_d = din("onescol", [128, 1], f16)
    ones_d = din("ones", [1, 128])
    eps_d = din("eps", [128, 1])

    risk_d = nc.dram_tensor("risk", [8, 1], f32, kind="ExternalOutput")
    if debug:
        xdbg_d = nc.dram_tensor("xdbg", [L, 128, NBLK * 128], f16,
                                kind="ExternalOutput")

    h_locs = [nc.dram_tensor(f"h_loc{i}", [NPAD, ROWC], u8)
              for i in range(2)]
    h_fulls = [nc.dram_tensor(f"h_full{i}", [N, ROWC], u8,
                              addr_space="Shared") for i in range(2)]
    rg = [list(range(NCORES))]

    from contextlib import ExitStack
    with tile.TileContext(nc) as tc, ExitStack() as es:
        if True:
            cpool = es.enter_context(tc.tile_pool(name="const", bufs=1))
            xpool = es.enter_context(tc.tile_pool(name="xbuf", bufs=1))
            bdp = es.enter_context(tc.tile_pool(name="bd", bufs=2))
            wk = es.enter_context(tc.tile_pool(name="wk", bufs=2))
            wkT = es.enter_context(tc.tile_pool(name="wkT", bufs=2))
            hsp = es.enter_context(tc.tile_pool(name="hst", bufs=2))
            hgp = es.enter_context(tc.tile_pool(name="hg", bufs=3))
            ibp = es.enter_context(tc.tile_pool(name="ib", bufs=2))
            relp = es.enter_context(tc.tile_pool(name="rel", bufs=3))
            exg = es.enter_context(tc.tile_pool(name="ex", bufs=2))
            hpp = es.enter_context(tc.tile_pool(name="hpbuf", bufs=2))
            idxp = es.enter_context(tc.tile_pool(name="idxs", bufs=3))
            pmxp = es.enter_context(tc.tile_pool(name="pmaxp", bufs=2))
            psm = es.enter_context(tc.tile_pool(name="ps_mm", bufs=1, space="PSUM"))
            psg = es.enter_context(tc.tile_pool(name="ps_agg", bufs=2, space="PSUM"))
            psa = es.enter_context(tc.tile_pool(name="ps_ald", bufs=2, space="PSUM"))
            psp = es.enter_context(tc.tile_pool(name="ps_pool", bufs=1, space="PSUM"))
            pso = es.enter_context(tc.tile_pool(name="ps_out", bufs=1, space="PSUM"))
            def load(dr):
                t = cpool.tile(list(dr.shape), dr.ap().dtype, tag=dr.name)
                nc.sync.dma_start(t[:], dr[:])
                return t

            x = xpool.tile([128, NBLK, 128], f16, tag="x")
            h_sb = xpool.tile([128, NBLK, 128], f16, tag="hsb")
            acsb = xpool.tile([128, NBLK, 8], f16, tag="acsb")
            nc.sync.dma_start(
                x[:], x0_d[:].rearrange("p (b f) -> p b f", f=128))
            iota, iotaT, ident16, ident8 = (load(iota_d), load(iotaT_d),
                                            load(ident16_d), load(ident8_d))
            onescol, ones, eps = load(onescol_d), load(ones_d), load(eps_d)
            ngb, hw, pw, rcnt = (load(ngb_d), load(hw_d), load(pw_d),
                                 load(rcnt_d))
            beta, betar, h0s, linb = (load(beta_d), load(betar_d),
                                      load(h0_d), load(linb_d))
            B2 = cpool.tile([128, NBLK, 2], f16, tag="B2")
            nc.sync.dma_start(B2[:],
                              B2_d[:].rearrange("p (b t) -> p b t", t=2))
            Wcat = cpool.tile([128, (L + 1) * WCOL], f16, tag="Wcat")
            for j in range(L + 1):
                nc.sync.dma_start(Wcat[:, j * WCOL:(j + 1) * WCOL], Wcat_d[j])
            linW = cpool.tile([128, (L + 1) * OUT], f32, tag="linW")
            for j in range(L + 1):
                nc.sync.dma_start(linW[:, j * OUT:(j + 1) * OUT], linW_d[j])
            cbs = None
            if not cb_zero:
                cbs = cpool.tile([128, L * 128], f16, tag="cbs")
                for j in range(L):
                    nc.sync.dma_start(cbs[:, j * 128:(j + 1) * 128], cb_d[j])

            out_acc = pso.tile([8, OUT], f32)

            def tt(out, in0, in1, op):
                nc.vector.tensor_tensor(out=out, in0=in0, in1=in1, op=op)

            def ts(out, in0, s1, op):
                nc.vector.tensor_scalar(out=out, in0=in0, scalar1=s1,
                                        scalar2=None, op0=op)

            def start_pool():
                pool_ps = psp.tile([128, 3 * GL], f32, tag="pool", bufs=1)
                nc.vector.memset(pool_ps[:], 0.0)
                pmax = pmxp.tile([128, GL], f32, tag="pmax")
                nc.vector.memset(pmax[:], -1e30)
                return pool_ps, pmax

            def phase0_block(l, b, BDl, pstate, h_loc_d):
                pool_ps, pmax = pstate
                xT = wkT.tile([128, 128], f16, tag="xT")
                if l == 0:
                    nc.sync.dma_start(xT[:], x0T_d[:, 128 * b:128 * (b + 1)])
                else:
                    tp = psm.tile([128, 128], f16, tag="mm", bufs=2)
                    nc.tensor.transpose(tp[:], x[:, b, :], ident16[:])
                    nc.scalar.copy(xT[:], tp[:])
                ph = psm.tile([128, WCOL], f32, tag="mm", bufs=2)
                nc.tensor.matmul(ph[:], lhsT=xT[:],
                                 rhs=Wcat[:, l * WCOL:(l + 1) * WCOL],
                                 start=True, stop=True)
                if l < L:
                    hst = hsp.tile([128, ROWC], u8, tag="hst")
                    nc.scalar.copy(hst[:, 0:128].bitcast(f8), ph[:, 0:128])
                    nc.scalar.activation(hst[:, 128:144].bitcast(f16),
                                         ph[:, 128:136], AT.Exp)
                    nc.scalar.copy(h_sb[:, b, :], ph[:, 0:128])
                    nc.vector.tensor_copy(acsb[:, b, :],
                                          hst[:, 128:144].bitcast(f16))
                    nc.scalar.activation(BDl[:, b, :], ph[:, 136:144],
                                         AT.Exp)
                    nc.sync.dma_start(h_loc_d[128 * b:128 * (b + 1), :],
                                      hst[:])
                # gate: sigmoid via exp table, then exp(sigmoid)
                u = wk.tile([128, 1], f32, tag="u")
                nc.scalar.activation(u[:], ph[:, 144:145], AT.Exp,
                                     bias=ngb[:, l:l + 1], scale=-1.0)
                ts(u[:], u[:], 1.0, OP.add)
                nc.vector.reciprocal(u[:], u[:])
                egt = wk.tile([128, 1], f16, tag="egt")
                nc.scalar.activation(egt[:], u[:], AT.Exp)
                egB2 = wk.tile([128, 4], f16, tag="egB2")
                tt(egB2[:, 0:2], B2[:, b, :],
                   egt[:].to_broadcast([128, 2]), OP.mult)
                nc.scalar.copy(egB2[:, 2:4], B2[:, b, :])
                lg = min(segs[b][0][2], GL - 2)
                nc.tensor.matmul(pool_ps[:, lg:lg + 2],
                                 lhsT=x[:, b, :], rhs=egB2[:, 0:2],
                                 start=False, stop=False,
                                 skip_group_check=True)
                nc.tensor.matmul(pool_ps[:, GL + lg:GL + lg + 2],
                                 lhsT=x[:, b, :], rhs=egB2[:, 2:4],
                                 start=False, stop=False,
                                 skip_group_check=True)
                nc.tensor.matmul(pool_ps[0:1, 2 * GL + lg:2 * GL + lg + 2],
                                 lhsT=onescol[:], rhs=egB2[:, 0:2],
                                 start=False, stop=False,
                                 skip_group_check=True)
                for (c0, c1, gseg) in segs[b]:
                    red = wk.tile([128, 1], f32, tag="red")
                    nc.vector.reduce_max(red[:], xT[:, c0:c1], axis=AX.X)
                    tt(pmax[:, gseg:gseg + 1], pmax[:, gseg:gseg + 1],
                       red[:], OP.max)

            def pool_epilogue(l, pstate):
                pool_ps, pmax = pstate
                ceg = wk.tile([1, GL], f32, tag="ceg")
                nc.scalar.copy(ceg[:], pool_ps[0:1, 2 * GL:3 * GL])
                nc.vector.reciprocal(ceg[:], ceg[:])
                bc_ps = psm.tile([128, GL], f32, tag="mm", bufs=2)
                nc.tensor.matmul(bc_ps[:], lhsT=ones[:], rhs=ceg[:],
                                 start=True, stop=True)
                hpw = wk.tile([128, 3], f32, tag="hpw")
                for j in range(3):
                    tt(hpw[:, j:j + 1], hw[:, l:l + 1], pw[:, j:j + 1],
                       OP.mult)
                bcs = wk.tile([128, GL], f32, tag="bcs")
                nc.scalar.copy(bcs[:], bc_ps[:])
                psb = wk.tile([128, GL], f32, tag="psb")
                tt(psb[:], pool_ps[:, 0:GL], bcs[:], OP.mult)
                ts(psb[:], psb[:], hpw[:, 0:1], OP.mult)
                t2 = wk.tile([128, GL], f32, tag="t2")
                tt(t2[:], pool_ps[:, GL:2 * GL], rcnt[:], OP.mult)
                ts(t2[:], t2[:], hpw[:, 1:2], OP.mult)
                tt(psb[:], psb[:], t2[:], OP.add)
                ts(t2[:], pmax[:], hpw[:, 2:3], OP.mult)
                tt(psb[:], psb[:], t2[:], OP.add)
                nc.tensor.matmul(out_acc[:], lhsT=psb[:],
                                 rhs=linW[:, l * OUT:(l + 1) * OUT],
                                 start=(l == 0), stop=(l == L),
                                 skip_group_check=True)

            def maybe_ag(l_next, b, h_loc_d, hf_d):
                # h_full is chunk-major: AllGather chunk c lands at rows
                # [8*r0, 8*r1) (rank-major within the chunk); host remaps
                # gather indices to match.
                if l_next >= L:
                    return
                for (cb0, cb1) in CHUNKS:
                    if cb1 - 1 == b:
                        r0, r1 = 128 * cb0, min(128 * cb1, NL)
                        nc.gpsimd.collective_compute(
                            "AllGather", OP.bypass, replica_groups=rg,
                            ins=[h_loc_d[r0:r1, :]],
                            outs=[hf_d[NCORES * r0:NCORES * r1, :]])

            # ---------------- layer 0 phase 0 ----------------
            BDl = bdp.tile([128, NBLK, 8], f16, tag="BD")
            pstate = start_pool()
            for b in range(NBLK):
                phase0_block(0, b, BDl, pstate, h_locs[0])
                maybe_ag(0, b, h_locs[0], h_fulls[0])
            pool_epilogue(0, pstate)

            # ---------------- fused layers ----------------
            def load_bucket(nmix, idx_d, rc_d, rbc_d, t0, TG, r0, r1,
                            hf, q):
                if TG == 0:
                    return None
                ist = idxp.tile([128, TG * 8], dt.int16, tag="i" + nmix)
                nc.sync.dma_start(ist[:], idx_d[:, t0 * 8:(t0 + TG) * 8])
                rc = relp.tile([128, TG], f16, tag="rc" + nmix)
                nc.sync.dma_start(rc[:], rc_d[:, t0:t0 + TG])
                rbc = relp.tile([128, TG, 128], f16, tag="rb" + nmix)
                nc.sync.dma_start(
                    rbc[:],
                    rbc_d[:, t0 * 128:(t0 + TG) * 128].rearrange(
                        "p (t e) -> p t e", e=128))
                gt = hgp.tile([128, TG, ROWC], u8, tag="h" + nmix)
                nc.gpsimd.dma_gather(
                    out_ap=gt[:], in_ap=hf[r0:r1, :], idxs_ap=ist[:],
                    num_idxs=TG * 128, num_idxs_reg=TG * 128,
                    elem_size=ROWC, single_packet=False, queue_num=q)
                Ib = ibp.tile([128, TG, 128], f16, tag="Ib" + nmix)
                tt(Ib[:],
                   bc(rc[:], [1, TG], [0, 128]),
                   bc(iota[:], [0, TG], [1, 128]),
                   OP.is_equal)
                IbT = ibp.tile([128, TG, 128], f16, tag="IbT" + nmix)
                tt(IbT[:], rbc[:], bc(iotaT[:], [0, TG], [1, 128]),
                   OP.is_equal)
                return (gt, Ib, IbT)

            def edge_bucket(nmix, bufn, Tb, goff0, koff, b, BDl, agg,
                            ald_ps, first):
                gt, Ib, IbT = bufn
                for t in range(Tb):
                    nc.tensor.matmul(
                        ald_ps[:, (koff + t) * 8:(koff + t + 1) * 8],
                        lhsT=IbT[:, goff0 + t, :], rhs=BDl[:, b, :],
                        start=True, stop=True, skip_group_check=True)
                ald_v = ald_ps[:].rearrange("p (t e) -> p t e", e=8)
                e18 = exg.tile([128, Tb, 8], f16, tag="e1" + nmix)
                tt(e18[:],
                   gt[:, goff0:goff0 + Tb, 128:144].bitcast(f16),
                   ald_v[:, koff:koff + Tb, 0:8], OP.mult)
                num = exg.tile([128, Tb, 4], f16, tag="num" + nmix)
                tt(num[:], e18[:, :, 0:4], e18[:, :, 4:8], OP.max)
                hp = hpp.tile([128, Tb, 132], f16, tag="hp" + nmix)
                nc.vector.tensor_tensor(
                    out=bc(hp[:, :, 0:128], [132, Tb], [32, 4], [1, 32]),
                    in0=bc(gt[:, goff0:goff0 + Tb, 0:128].bitcast(f8),
                           [ROWC, Tb], [32, 4], [1, 32]),
                    in1=bc(num[:], [4, Tb], [1, 4], [0, 32]),
                    op=OP.mult)
                nc.scalar.copy(hp[:, :, 128:132], num[:])
                for t in range(Tb):
                    nc.tensor.matmul(
                        agg[:], lhsT=Ib[:, goff0 + t, :],
                        rhs=hp[:, t, :], start=first,
                        stop=False, skip_group_check=True)
                    first = False
                return first

            def edge_block(l, b, g0, a0, b0, buf, BDl):
                TAB = TA[b] + TB[b]
                agg = psg.tile([128, 132], f32, tag="agg")
                ald_ps = psa.tile([128, TAB * 8], f32, tag="ald")
                first = True
                koff = 0
                for nmix, Tb, goff0 in (("A", TA[b], int(aoff[b]) - a0),
                                        ("B", TB[b], int(boff[b]) - b0)):
                    if Tb == 0 or buf[nmix] is None:
                        continue
                    first = edge_bucket(nmix, buf[nmix], Tb, goff0, koff,
                                        b, BDl, agg, ald_ps, first)
                    koff += Tb
                nS8 = wk.tile([128, 8], f16, tag="nS8")
                tt(nS8[:], acsb[:, b, :], BDl[:, b, :], OP.mult)
                nS = wk.tile([128, 4], f16, tag="nS")
                tt(nS[:], nS8[:, 0:4], nS8[:, 4:8], OP.max)
                den = wk.tile([128, 4], f32, tag="den")
                ts(den[:], agg[:, 128:132], eps[:, 0:1], OP.add)
                tt(den[:], den[:], nS[:], OP.add)
                nc.vector.reciprocal(den[:], den[:])
                t1 = wk.tile([128, 128], f16, tag="t1")
                nc.vector.tensor_tensor(
                    out=bc(t1[:], [32, 4], [1, 32]),
                    in0=bc(h_sb[:, b, :], [32, 4], [1, 32]),
                    in1=bc(nS[:], [1, 4], [0, 32]), op=OP.mult)
                tt(bc(t1[:], [32, 4], [1, 32]),
                   bc(t1[:], [32, 4], [1, 32]),
                   bc(agg[:, 0:128], [32, 4], [1, 32]), OP.add)
                tt(bc(t1[:], [32, 4], [1, 32]),
                   bc(t1[:], [32, 4], [1, 32]),
                   bc(den[:], [1, 4], [0, 32]), OP.mult)
                if not cb_zero:
                    tt(t1[:], t1[:],
                       cbs[:, l * 128:(l + 1) * 128], OP.add)
                if l < L - 1:
                    nc.scalar.activation(x[:, b, :], t1[:], AT.Relu)
                else:
                    nc.scalar.copy(x[:, b, :], t1[:])

            for l in range(L):
                hf = h_fulls[l % 2]
                h_loc_next = h_locs[(l + 1) % 2]
                hf_next = h_fulls[(l + 1) % 2]
                BDn = None
                if l + 1 < L:
                    BDn = bdp.tile([128, NBLK, 8], f16, tag="BD")
                pstate = start_pool()
                for gi, (g0, g1) in enumerate(groups):
                    a0, a1 = int(aoff[g0]), int(aoff[g1])
                    b0, b1 = int(boff[g0]), int(boff[g1])
                    buf = {
                        "A": load_bucket("A", idxA_d, rcA_d, rbcA_d, a0,
                                         a1 - a0, 0, BKT, hf,
                                         (2 * gi) % 4),
                        "B": load_bucket("B", idxB_d, rcB_d, rbcB_d, b0,
                                         b1 - b0, BKT, N, hf,
                                         (2 * gi + 1) % 4),
                    }
                    for b in range(g0, g1):
                        edge_block(l, b, g0, a0, b0, buf, BDl)
                        phase0_block(l + 1, b, BDn, pstate, h_loc_next)
                        maybe_ag(l + 1, b, h_loc_next, hf_next)
                BDl = BDn
                if debug:
                    nc.sync.dma_start(
                        xdbg_d[l].rearrange("p (b f) -> p b f", f=128),
                        x[:])
                pool_epilogue(l + 1, pstate)

            # ---------------- final risk ----------------
            oa = wk.tile([8, OUT], f32, tag="oa")
            nc.scalar.copy(oa[:], out_acc[:])
            oT_ps = psm.tile([64, 8], f32, tag="mm", bufs=2)
            nc.tensor.transpose(oT_ps[:], oa[:], ident8[:])
            oT = wk.tile([64, 8], f32, tag="oT")
            nc.scalar.copy(oT[:], oT_ps[:])
            risk_ps = psm.tile([8, 1], f32, tag="mm", bufs=2)
            nc.tensor.matmul(risk_ps[:], lhsT=oT[:], rhs=beta[:],
                             start=True, stop=True)
            blb = wk.tile([1, OUT], f32, tag="blb")
            ts(blb[:], linb[:, 0:OUT], hw[0:1, 0:1], OP.mult)
            tmp = wk.tile([1, OUT], f32, tag="tmpb")
            for j in range(1, L + 1):
                ts(tmp[:], linb[:, j * OUT:(j + 1) * OUT],
                   hw[0:1, j:j + 1], OP.mult)
                tt(blb[:], blb[:], tmp[:], OP.add)
            tt(tmp[:], blb[:], betar[:], OP.mult)
            csc = wk.tile([1, 1], f32, tag="csc")
            nc.vector.reduce_sum(csc[:], tmp[:], axis=AX.X)
            tt(csc[:], csc[:], h0s[:], OP.add)
            c_ps = psm.tile([8, 1], f32, tag="mm", bufs=2)
            nc.tensor.matmul(c_ps[:], lhsT=ones[:, 0:8], rhs=csc[:],
                             start=True, stop=True)
            rsb = wk.tile([8, 1], f32, tag="rsb")
            nc.vector.tensor_copy(rsb[:], risk_ps[:])
            tt(rsb[:], rsb[:], c_ps[:], OP.add)
            nc.sync.dma_start(risk_d[:], rsb[:])

    nc.compile()
    return nc


# ----------------------------------------------------------------------
# host inputs
# ----------------------------------------------------------------------
def _host_inputs(inputs, cores, TA, TB, bounds, counts):
    x = np.asarray(inputs["x"], np.float32)
    Ws = np.asarray(inputs["Ws"], np.float64)
    att_src = np.asarray(inputs["att_src"], np.float64)
    att_dst = np.asarray(inputs["att_dst"], np.float64)
    conv_b = np.asarray(inputs["conv_b"], np.float32)
    gate_W = np.asarray(inputs["gate_W"], np.float64)
    gate_b = np.asarray(inputs["gate_b"], np.float32)
    lin_W = np.asarray(inputs["lin_W"], np.float32)
    lin_b = np.asarray(inputs["lin_b"], np.float32)
    h_weights = np.asarray(inputs["h_weights"], np.float32)
    h0 = np.asarray(inputs["h0"], np.float32)
    beta = np.asarray(inputs["beta"], np.float32)
    pool_w = np.asarray(inputs["pool_w"], np.float32)

    Wcat = np.zeros((L + 1, 128, WCOL), np.float64)
    for l in range(L):
        Wcat[l, :, 0:128] = Ws[l]
        for h in range(H):
            ac_s = np.zeros(HC); ac_s[h * C:(h + 1) * C] = att_src[l, h]
            ac_d = np.zeros(HC); ac_d[h * C:(h + 1) * C] = att_dst[l, h]
            wa_s = Ws[l] @ ac_s          # [128]
            wa_d = Ws[l] @ ac_d
            Wcat[l, :, 128 + h] = wa_s
            Wcat[l, :, 132 + h] = 0.2 * wa_s
            Wcat[l, :, 136 + h] = wa_d
            Wcat[l, :, 140 + h] = 0.2 * wa_d
    for l in range(L + 1):
        Wcat[l, :, 144] = gate_W[l][:, 0]

    shared = dict(
        Wcat=Wcat.astype(np.float16),
        ngb=np.tile((-gate_b)[None, :], (128, 1)).astype(np.float32),
        cb=np.tile(conv_b[:, None, :], (1, 128, 1)).astype(np.float16),
        linW=lin_W.astype(np.float32),
        linb=lin_b.reshape(1, -1).astype(np.float32),
        hw=np.tile(h_weights[None, :], (128, 1)).astype(np.float32),
        pw=np.tile(pool_w[None, :], (128, 1)).astype(np.float32),
        beta=beta.reshape(64, 1).astype(np.float32),
        betar=beta.reshape(1, 64).astype(np.float32),
        h0=h0.reshape(1, 1).astype(np.float32),
        iota=np.tile(np.arange(128, dtype=np.float16)[None, :], (128, 1)),
        iotaT=np.tile(np.arange(128, dtype=np.float16)[:, None], (1, 128)),
        ident16=np.eye(128, dtype=np.float16),
        ident8=np.eye(8, dtype=np.float32),
        onescol=np.ones((128, 1), np.float16),
        ones=np.ones((1, 128), np.float32),
        eps=np.full((128, 1), 1e-16, np.float32),
    )

    segs = _chunk_segments(bounds)
    bt_local = (np.arange(NL) * G // N).astype(np.int64)
    in_maps = []
    for m in range(NCORES):
        c = cores[m]
        xm = np.zeros((NPAD, 128), np.float32)
        xm[:NL] = x[m * NL:(m + 1) * NL]
        x0p = np.ascontiguousarray(
            xm.reshape(NBLK, 128, 128).transpose(1, 0, 2)
        ).reshape(128, NBLK * 128).astype(np.float16)
        x0T = np.ascontiguousarray(
            xm.reshape(NBLK, 128, 128).transpose(2, 0, 1)
        ).reshape(128, NBLK * 128).astype(np.float16)
        B2 = np.zeros((128, NBLK, 2), np.float16)
        for k in range(NBLK):
            lg = min(segs[k][0][2], GL - 2)
            for p in range(128):
                n = 128 * k + p
                if n < NL:
                    g = bt_local[n]
                    if g == lg:
                        B2[p, k, 0] = 1.0
                    elif g == lg + 1:
                        B2[p, k, 1] = 1.0
        rcnt = np.tile((1.0 / counts[m * GL:(m + 1) * GL])[None, :],
                       (128, 1)).astype(np.float32)

        def rel_col(r):
            rr = np.where(r < 0, 255, r).astype(np.float16)
            return np.ascontiguousarray(rr.reshape(-1, 128).T)

        def rel_bc(r):
            rr = np.where(r < 0, 255, r).astype(np.float16)
            return np.ascontiguousarray(np.tile(rr[None, :], (128, 1)))

        in_maps.append(dict(
            x0=x0p, x0T=x0T,
            idxA=_wrap_idxs(c["idxA"]), idxB=_wrap_idxs(c["idxB"]),
            rcA=rel_col(c["relA"]), rcB=rel_col(c["relB"]),
            rbcA=rel_bc(c["relA"]), rbcB=rel_bc(c["relB"]),
            B2=np.ascontiguousarray(B2).reshape(128, NBLK * 2),
            rcnt=rcnt,
            **shared,
        ))
    return in_maps


# ----------------------------------------------------------------------
# entry point
# ----------------------------------------------------------------------
def kernel(**inputs):
    global LAST_RESULTS
    from concourse.bass_utils import run_bass_kernel_spmd

    trace = bool(int(os.environ.get("KERNEL_TRACE", "0")))
    debug = bool(int(os.environ.get("KERNEL_DEBUG", "0")))
    if trace:
        try:
            from trn_agent_boot.trn_boot import _ntff_profile_via_ctypes
            if "antenv.axon_hooks" not in sys.modules:
                _m = types.ModuleType("antenv.axon_hooks")
                _hook = _ntff_profile_via_ctypes("/opt/axon/libaxon_pjrt.so")
                _m.get_axon_ntff_profile_hook = lambda: _hook
                sys.modules["antenv.axon_hooks"] = _m
        except Exception:
            trace = False

    src = np.asarray(inputs["src"])
    dst = np.asarray(inputs["dst"])
    batch = np.asarray(inputs["batch"])
    cb_zero = not np.asarray(inputs["conv_b"]).any()
    key = (src.tobytes(), dst.tobytes(), batch.tobytes(), cb_zero, debug)
    if key not in _cache:
        cores, TA, TB, bounds, counts = _prep(src, dst, batch)
        nc = _build(TA, TB, bounds, cb_zero, debug=debug)
        _cache[key] = (nc, cores, TA, TB, bounds, counts)
    nc, cores, TA, TB, bounds, counts = _cache[key]

    in_maps = _host_inputs(inputs, cores, TA, TB, bounds, counts)
    res = run_bass_kernel_spmd(nc, in_maps, list(range(NCORES)),
                               trace=trace)
    LAST_RESULTS = res
    risk = np.concatenate(
        [res.results[m]["risk"][:, 0] for m in range(NCORES)])
    return risk.astype(np.float32)
